# revision 8
# baseline (speedup 1.0000x reference)
"""FLAttention Bass/Tile kernel for Trainium2, batch-sharded over 8 NeuronCores.

Math per (head h, batch row b), with x_b = x[b, :] (D=512):
    q_j = a_q x_j + b_q ; k_i = a_k x_i + b_k ; v_j = a_v x_j + b_v
    u[i,j] = |q_j - k_i| + eps
    w[i,j] = softmax_j(1/u[i,j])                 (stabilized by row max)
    att[i] = sum_j w[i,j] v_j / sqrt(D)
    out = x + sum_h att_h

Per-core implementation (8 batch rows per core, all 8 heads), i (output
row) on SBUF partitions (4 chunks of 128), j on free dim (512):
  - POOL: z = a_q x_j + (b_q - b_k - a_k x_i) per chunk (per-partition
    scalar affine; POOL has no other work and runs well under the budget).
  - DVE: ONE fused custom op per chunk: r = -1/max(z, eps-z) (that inner
    max is |z| with an eps floor), seeded by the BITWISE_NOT exponent
    trick + one Newton step (~0.2% rel err), with a fused MIN-accumulator
    giving the exp stabilizer bias -m = min_j r exactly consistent with
    the r values, so the top softmax element cancels to exp(0) exactly.
  - ACT: Exp(scale=-1, bias=-m) with fused row-sum -> e (bf16), S.  ACT
    runs nothing else; it is the bottleneck engine at its floor.
  - DVE: scalar_tensor_tensor (e * a_v) * x with fused f32 row-sum
    accumulator, all-bf16 operands -> 4x DVE mode.  v is never
    materialized: sum_j e v_j = a_v * sum_j e x_j + b_v * S.
  - Batched epilogue: att = sum_h (a_v SVx)/S + sum_h b_v,
    out = x + att/sqrt(D).
  The pair loop is software-pipelined (lookahead 2) so ACT streams
  back-to-back exps while POOL/DVE prepare the next pair.
"""

import os

import numpy as np

import concourse.bacc as bacc
import concourse.bass as bass
import concourse.dve_ops as dve_ops
import concourse.mybir as mybir
import concourse.tile as tile
from concourse.bass_utils import run_bass_kernel_spmd
from concourse.dve_spec import AluOp, Bin, C0, C1, C2, One, Spec, Src0, lower, maxx
from concourse.dve_uop import DveOpSpec


def _register_absrecip():
    """ABS_NEG_RECIP_ANT: out = -1/max(x, c2 - x) at ~0.2% rel err, with
    accum_out = min(out).  max(x, eps-x) == |x| with an eps/2 floor (and
    |x|+eps on the negative side), so the reciprocal never sees zero.
    Seed y0 = BITWISE_NOT(u)*c0 ~= +1/u; one Newton step with the sign
    folded into the final multiply: out = y0*(u*y0 - c1) = -y0*(c1 - u*y0).
    c0/c1 are the tuned RECIPROCAL_APPROX_FAST constants."""
    name = "ABS_NEG_RECIP_ANT"
    for op in dve_ops.OPS:
        if op.name == name:
            return op
    u = maxx(Src0, C2 - Src0)
    nu = Bin(AluOp.BITWISE_NOT, u, u)
    y0 = nu * C0
    t = u * y0
    w = t - C1
    body = y0 * w

    def _ref(in0, in1, c0, c1, c2):
        x = in0.astype(np.float32)
        uu = np.maximum(x, (np.float32(c2) - x).astype(np.float32)).astype(
            np.float32
        )
        nuu = (~uu.view(np.int32)).view(np.float32)
        yy0 = (nuu * np.float32(c0)).astype(np.float32)
        tt = (uu * yy0).astype(np.float32)
        ww = (tt - np.float32(c1)).astype(np.float32)
        out = (yy0 * ww).astype(np.float32)
        acc = np.minimum(np.float32(1.0), out.min(axis=-1, keepdims=True))
        return out, acc

    spec = Spec(body=body, reference=_ref, accum=AluOp.MIN, accum_init=One)
    op = dve_ops.DveOp(name, spec, subdim=False, uops_sha={})
    dve_ops.OPS.append(op)
    dve_ops.CUSTOM_DVE_SPECS[name] = op.spec
    dve_ops._SUB_OPCODE_FOR_NAME[name] = (
        max(dve_ops._SUB_OPCODE_FOR_NAME.values()) + 1
    )
    shas = {}
    for ver in ("v3", "v4"):
        ospec = DveOpSpec(
            name=name,
            opcode=dve_ops.get_dve_sub_opcode(name),
            uops=lower(op.spec, ver=ver),
            rd1_en=False,
        )
        shas[ver] = ospec.sha(ver)
    object.__setattr__(op, "uops_sha", shas)
    return op


ABSRECIP = _register_absrecip()
RC = dve_ops.RECIP_APPROX_FAST_CONSTS

B, D, H = 64, 512, 8
NCORES = 8
RB = B // NCORES          # batch rows per core
NCH = D // 128            # 4 partition chunks of the i dimension
EPS = 1e-8
INV_SQRT_D = float(np.float32(1.0) / np.sqrt(np.float32(D)))

F32 = mybir.dt.float32
BF16 = mybir.dt.bfloat16
AF = mybir.ActivationFunctionType
OP = mybir.AluOpType
AX = mybir.AxisListType

# tuning knobs (read once at build time)
ZDVE = int(os.environ.get("KBENCH_ZDVE", "0"))      # z chunks on DVE (rest POOL)
LOOKAHEAD = int(os.environ.get("KBENCH_LOOKAHEAD", "2"))
WBUFS = int(os.environ.get("KBENCH_WBUFS", "8"))
PBUFS = int(os.environ.get("KBENCH_PBUFS", "3"))


def build_nc():
    nc = bacc.Bacc(
        "TRN2",
        target_bir_lowering=False,
        debug=False,
        num_devices=NCORES,
    )
    x_d = nc.dram_tensor("x_shard", [RB, D], F32, kind="ExternalInput")
    al_d = nc.dram_tensor("alphas", [H, 3], F32, kind="ExternalInput")
    be_d = nc.dram_tensor("betas", [H, 3], F32, kind="ExternalInput")
    out_d = nc.dram_tensor("out_shard", [RB, D], F32, kind="ExternalOutput")

    with tile.TileContext(nc) as tc:
        with (
            tc.tile_pool(name="const", bufs=1) as cpool,
            tc.tile_pool(name="work", bufs=WBUFS) as wpool,
            tc.tile_pool(name="pair", bufs=PBUFS) as rpool,
            tc.tile_pool(name="junkp", bufs=2) as jpool,
            tc.tile_pool(name="psum", bufs=2, space="PSUM") as ppool,
        ):
            # ---------------- prologue ----------------
            ones = cpool.tile([1, 128], F32, tag="ones")
            nc.vector.memset(ones[:], 1.0)

            x_flat = cpool.tile([1, RB * D], F32, tag="xflat")
            nc.sync.dma_start(
                x_flat[:], x_d.ap().rearrange("(o b) d -> o (b d)", o=1)
            )

            # alphas/betas as a single row [1, 48]: cols h*3+t, 24 + h*3+t
            ab_row = cpool.tile([1, 48], F32, tag="abrow")
            nc.sync.dma_start(
                ab_row[:, 0:24], al_d.ap().rearrange("(o h) t -> o (h t)", o=1)
            )
            nc.sync.dma_start(
                ab_row[:, 24:48], be_d.ap().rearrange("(o h) t -> o (h t)", o=1)
            )

            # broadcast scalars across partitions via PE outer product
            ab_psum = ppool.tile([128, 48], F32, tag="abp")
            nc.tensor.matmul(ab_psum[:], ones[:], ab_row[:])
            AB = cpool.tile([128, 48], F32, tag="ab")
            nc.vector.tensor_copy(AB[:], ab_psum[:])

            # x in partition-major layout: X_part[p, b*NCH + c] = x[b, c*128+p]
            x_part = cpool.tile([128, RB * NCH], F32, tag="xpart")
            nc.sync.dma_start(
                x_part[:], x_d.ap().rearrange("b (c p) -> p (b c)", p=128)
            )

            # negak[:, h] = -a_k[h], bqk[:, h] = b_q[h] - b_k[h]
            ab3 = AB[:, 0:24].rearrange("p (h t) -> p h t", t=3)
            bb3 = AB[:, 24:48].rearrange("p (h t) -> p h t", t=3)
            negak = cpool.tile([128, H], F32, tag="negak")
            nc.vector.tensor_scalar_mul(negak[:], ab3[:, :, 1], -1.0)
            bqk = cpool.tile([128, H], F32, tag="bqk")
            nc.vector.tensor_sub(bqk[:], bb3[:, :, 0], bb3[:, :, 1])

            # KK[:, h*RB*NCH + b*NCH + c] = b_q - b_k - a_k x_i for that column
            KK = cpool.tile([128, H * RB * NCH], F32, tag="kk")
            for h in range(H):
                nc.vector.tensor_scalar(
                    KK[:, h * RB * NCH : (h + 1) * RB * NCH],
                    x_part[:],
                    negak[:, h : h + 1],
                    bqk[:, h : h + 1],
                    op0=OP.mult,
                    op1=OP.add,
                )

            # X broadcast tiles in SBUF (PE outer product via PSUM, then
            # copy), plus a bf16 copy for the 4x-mode SVx accumulation.
            xb_sbuf = []
            xb_bf = []
            for b in range(RB):
                pt = ppool.tile([128, D], F32, tag=f"xb{b % 2}")
                nc.tensor.matmul(pt[:], ones[:], x_flat[:, b * D : (b + 1) * D])
                st = cpool.tile([128, D], F32, tag=f"xs{b}")
                nc.vector.tensor_copy(st[:], pt[:])
                sb = cpool.tile([128, D], BF16, tag=f"xsb{b}")
                nc.vector.tensor_copy(sb[:], pt[:])
                xb_sbuf.append(st)
                xb_bf.append(sb)

            # stats: col = (b*NCH + c)*H + h
            NST = RB * NCH * H
            S_all = cpool.tile([128, NST], F32, tag="sall")
            SVx_all = cpool.tile([128, NST], F32, tag="svxall")

            # ---------------- main loop ----------------
            pair_state = {}

            def emit_front(b, h):
                z_big = wpool.tile([128, NCH * D], F32, tag="z")
                for c in range(NCH):
                    kcol = h * RB * NCH + b * NCH + c
                    eng = nc.vector if c < ZDVE else nc.gpsimd
                    eng.tensor_scalar(
                        z_big[:, c * D : (c + 1) * D],
                        xb_sbuf[b][:],
                        AB[:, 3 * h : 3 * h + 1],
                        KK[:, kcol : kcol + 1],
                        op0=OP.mult,
                        op1=OP.add,
                    )
                # r = -1/max(|z|, eps'); negm[c] = min_j r = -m (exp bias)
                r_big = rpool.tile([128, NCH * D], F32, tag="r")
                negm4 = rpool.tile([128, NCH], F32, tag="negm")
                for c in range(NCH):
                    nc.vector._custom_dve(
                        ABSRECIP,
                        out=r_big[:, c * D : (c + 1) * D],
                        in0=z_big[:, c * D : (c + 1) * D],
                        s0=RC["s0"],
                        s1=RC["s1"],
                        imm2=EPS,
                        accum_out=negm4[:, c : c + 1],
                    )
                pair_state[(b, h)] = (r_big, negm4)

            def emit_back(b, h):
                r_big, negm4 = pair_state.pop((b, h))
                for c in range(NCH):
                    scol = (b * NCH + c) * H + h
                    e_t = wpool.tile([128, D], BF16, tag="e")
                    nc.scalar.activation(
                        e_t[:],
                        r_big[:, c * D : (c + 1) * D],
                        AF.Exp,
                        scale=-1.0,
                        bias=negm4[:, c : c + 1],
                        accum_out=S_all[:, scol : scol + 1],
                    )
                    prod = wpool.tile([128, D], BF16, tag="prod")
                    nc.gpsimd.tensor_tensor(
                        prod[:], e_t[:], xb_bf[b][:], op=OP.mult
                    )
                    junk = jpool.tile([128, D], BF16, tag="junk")
                    nc.vector.tensor_scalar(
                        junk[:],
                        prod[:],
                        0.0,
                        None,
                        op0=OP.add,
                        op1=OP.add,
                        accum_out=SVx_all[:, scol : scol + 1],
                    )

            pairs = [(b, h) for b in range(RB) for h in range(H)]

            def emit_main():
                for i in range(len(pairs) + LOOKAHEAD):
                    if i < len(pairs):
                        emit_front(*pairs[i])
                    if i >= LOOKAHEAD and i - LOOKAHEAD < len(pairs):
                        emit_back(*pairs[i - LOOKAHEAD])

            # bench-only: repeat the (idempotent) main loop on-device so the
            # kernel time can be read off a wall-clock slope over repeats.
            repeat = int(os.environ.get("KBENCH_REPEAT", "1"))
            if repeat > 1:
                with tc.For_i(0, repeat, 1):
                    emit_main()
            else:
                emit_main()

            # ---------------- epilogue ----------------
            # att = sum_h (a_v SVx)/S + sum_h b_v ; out = x + att/sqrt(D)
            s_inv = cpool.tile([128, NST], F32, tag="sinv")
            nc.vector.reciprocal(s_inv[:], S_all[:])
            z2 = cpool.tile([128, NST], F32, tag="z2")
            nc.vector.tensor_mul(z2[:], SVx_all[:], s_inv[:])
            # a_v was left out of the SVx accumulation (an AP scalar would
            # break the 4x DVE mode); apply it per-head here via a
            # stride-0 broadcast view of AB over the (b,c) groups.
            av_view = (
                ab3[:, :, 2]
                .rearrange("p h -> p () h")
                .broadcast_to([128, RB * NCH, H])
            )
            z2_3d = z2[:].rearrange("p (g h) -> p g h", h=H)
            nc.vector.tensor_tensor(z2_3d, z2_3d, av_view, op=OP.mult)
            att = cpool.tile([128, RB * NCH], F32, tag="att")
            nc.vector.tensor_reduce(
                att[:],
                z2[:].rearrange("p (g h) -> p g h", h=H),
                axis=AX.X,
                op=OP.add,
            )
            bsum = cpool.tile([128, 1], F32, tag="bsum")
            nc.vector.tensor_reduce(
                bsum[:], bb3[:, :, 2], axis=AX.X, op=OP.add
            )
            t_sc = cpool.tile([128, RB * NCH], F32, tag="tsc")
            nc.vector.tensor_scalar(
                t_sc[:],
                att[:],
                bsum[:, 0:1],
                INV_SQRT_D,
                op0=OP.add,
                op1=OP.mult,
            )
            out_c = cpool.tile([128, RB * NCH], F32, tag="outc")
            nc.vector.tensor_add(out_c[:], t_sc[:], x_part[:])
            nc.sync.dma_start(
                out_d.ap().rearrange("b (c p) -> p b c", p=128),
                out_c[:].rearrange("p (b c) -> p b c", c=NCH),
            )

    nc.compile()
    return nc


_NC_CACHE = None


def get_nc():
    global _NC_CACHE
    if _NC_CACHE is None:
        _NC_CACHE = build_nc()
    return _NC_CACHE


def kernel(x: np.ndarray, alphas: np.ndarray, betas: np.ndarray) -> np.ndarray:
    x = np.ascontiguousarray(x, dtype=np.float32)
    alphas = np.ascontiguousarray(alphas, dtype=np.float32)
    betas = np.ascontiguousarray(betas, dtype=np.float32)

    nc = get_nc()
    in_maps = [
        {
            "x_shard": x[c * RB : (c + 1) * RB],
            "alphas": alphas,
            "betas": betas,
        }
        for c in range(NCORES)
    ]
    res = run_bass_kernel_spmd(nc, in_maps, core_ids=list(range(NCORES)))
    out = np.concatenate([res.results[c]["out_shard"] for c in range(NCORES)], axis=0)
    return out


if __name__ == "__main__":
    rng = np.random.default_rng(0)
    x = rng.standard_normal((B, D), dtype=np.float32)
    al = rng.random((H, 3), dtype=np.float32)
    be = rng.random((H, 3), dtype=np.float32)
    out = kernel(x=x, alphas=al, betas=be)
    print("out", out.shape, out.dtype, float(np.abs(out).max()))
